# revision 2
# baseline (speedup 1.0000x reference)
"""AttentionalPropagation Trainium2 Bass kernel, v2.

Reference (B=4, D=256, N=M=2048, H=4, head_dim=64):
    q = Wq@x+bq ; k = Wk@src+bk ; v = Wv@src+bv      (conv1x1)
    scores[b,h,n,m] = (q_h . k_h)/8
    prob = softmax_m(scores) * edge
    msg = prob @ v_h -> Wm -> out = W2@relu(W1@[x; msg])

Sharding: 8 cores = (batch 0..3) x (query-half 0..1); NQ=1024/core.

v2 changes vs v1 (all measured on HW):
  - q/k/v projections in fp8e4 DoubleRow (x/src/W quantized on host);
    v is produced PRE-TRANSPOSED (stationary=src-tile, moving=WvT) so the
    64 PE transposes + copies of v1 are gone.  v bias is folded into the
    PSUM->SBUF copy as a tensor_add with a host-broadcast bias tile.
  - ACT does exp ONLY (the 66us/core wall); q/k copies moved to DVE,
    v copies / MLP biases / relu / out biases to Pool.
  - softmax denominator: DVE pair-adds (8/hc) then ONE chained-ones
    PE matmul into dnb[0:1,:] (replaces v1's VE chain + ones matmul).
  - edge-mul writes u_e to a separate tile (no in-place WAR), split
    DVE (14 tiles) / Pool (2 tiles) per head-chunk.
  - msg matmuls (fp16 16-chain) are emitted one score-group late so the
    PE never stalls waiting on DVE.
  - final normalize reads the reciprocal broadcast directly from PSUM.
  - MLP runs 512-wide (half the group count of v1's 256-wide).
"""

import os
import numpy as np
import ml_dtypes

import concourse.bass as bass
import concourse.bacc as bacc
import concourse.mybir as mybir
import concourse.tile as tile
from concourse import bass_utils

F32 = mybir.dt.float32
F16 = mybir.dt.float16
F8 = mybir.dt.float8e4
AF = mybir.ActivationFunctionType
DR = mybir.MatmulPerfMode.DoubleRow

B, D, N, H = 4, 256, 2048, 4
HD = D // H          # 64
P = 128
NQ = N // 2          # 1024 queries per core
NCORES = 8
NMT = N // P         # 16 m-tiles

# fp8 weight block columns: Wq | Wk | Wv  (each [128, 2, 256])
W8_COLS = 3 * 512
# fp16 weight block columns: W1a | Wm | W1b | W2
OFF_W1A, OFF_WM, OFF_W1B, OFF_W2 = 0, 1024, 1536, 2560
W16_COLS = 3584
GROUPS = ((0, 3), (3, 3), (6, 3), (9, 3), (12, 3), (15, 1))
POOL_GROUP = 3                 # edge-mul group index handled by Pool

LAST_RESULTS = None  # test harness reads this


def build_program(reps: int = 1, ablate: str = 'full'):
    nc = bacc.Bacc(None, target_bir_lowering=False)

    wpk8 = nc.dram_tensor("wpk8", [P, W8_COLS], F8, kind="ExternalInput")
    xpk8 = nc.dram_tensor("xpk8", [P, 2 * NQ], F8, kind="ExternalInput")
    spk8 = nc.dram_tensor("spk8", [P, 2 * N], F8, kind="ExternalInput")
    wpk = nc.dram_tensor("wpk", [P, W16_COLS], F16, kind="ExternalInput")
    xpk = nc.dram_tensor("xpk", [P, 2 * NQ], F16, kind="ExternalInput")
    edgeT = nc.dram_tensor("edgeT", [N, NQ], F16, kind="ExternalInput")
    bvbc = nc.dram_tensor("bvbc", [P, D], F16, kind="ExternalInput")
    bpk = nc.dram_tensor("bpk", [P, 14], F32, kind="ExternalInput")
    out = nc.dram_tensor("out", [D, NQ], F32, kind="ExternalOutput")

    with tile.TileContext(nc) as tc:
        _loop = tc.For_i(0, reps, 1) if reps > 1 else None
        if _loop is not None:
            _loop.__enter__()
        with (
            tc.tile_pool(name="const", bufs=1) as cp,
            tc.tile_pool(name="w", bufs=1) as wp,
            tc.tile_pool(name="acts", bufs=1) as ap,
        ):
            ones16 = cp.tile([P, 1], F16)
            nc.vector.memset(ones16, 1.0)
            ones_row = cp.tile([1, HD], F16)
            nc.vector.memset(ones_row, 1.0)
            bias = cp.tile([P, 14], F32)
            bvb = cp.tile([P, D], F16)

            x8_sb = wp.tile([P, 2, NQ], F8)
            w8_sb = wp.tile([P, 3, 2, D], F8)     # [wq|wk|wv][kk][256]
            s8_sb = wp.tile([P, 2, N], F8)
            wx_sb = wp.tile([P, W16_COLS + 2 * NQ], F16)

            nc.sync.dma_start(out=x8_sb[:, :, :],
                              in_=xpk8[:, :].rearrange("p (k c) -> p k c", k=2))
            nc.sync.dma_start(out=w8_sb[:, :, :, :],
                              in_=wpk8[:, :].rearrange(
                                  "p (w k c) -> p w k c", w=3, k=2))
            nc.sync.dma_start(out=s8_sb[:, :, :],
                              in_=spk8[:, :].rearrange("p (k c) -> p k c", k=2))
            nc.sync.dma_start(out=bias[:, :], in_=bpk[:, :])
            nc.sync.dma_start(out=bvb[:, :], in_=bvbc[:, :])

            def wview(off, ncols, nk):
                return wx_sb[:, off:off + nk * ncols].rearrange(
                    "p (k c) -> p k c", k=nk)

            w1a_sb = wview(OFF_W1A, 2 * D, 2)
            wm_sb = wview(OFF_WM, D, 2)
            w1b_sb = wview(OFF_W1B, 2 * D, 2)
            w2_sb = wview(OFF_W2, D, 4)
            x_sb = wview(W16_COLS, NQ, 2)

            q_sb = ap.tile([P, 2, NQ], F16)
            k_sb = ap.tile([P, 2, N], F16)
            vt_sb = ap.tile([P, NMT, H, HD], F16)   # [m, mt, h, d]
            msg_sb = ap.tile([P, 2, NQ], F16)
            msg2_sb = ap.tile([P, 2, NQ], F16)
            h1_sb = ap.tile([P, 4, NQ], F16)

            # ---- phase 1: projections (fp8 DoubleRow) ----
            with tc.tile_pool(name="pp", bufs=2, space="PSUM") as pp:
                for dt_ in range(2):
                    for nchk in range(2):
                        ps = pp.tile([P, 512], F32, tag="ps")
                        nc.tensor.matmul(
                            ps[:, :],
                            w8_sb[:, 0, :, dt_ * P:(dt_ + 1) * P],
                            x8_sb[:, :, nchk * 512:(nchk + 1) * 512],
                            start=True, stop=True, perf_mode=DR)
                        nc.vector.tensor_scalar_add(
                            q_sb[:, dt_, nchk * 512:(nchk + 1) * 512], ps[:, :],
                            bias[:, dt_:dt_ + 1])
                for dt_ in range(2):
                    for nchk in range(4):
                        ps = pp.tile([P, 512], F32, tag="ps")
                        nc.tensor.matmul(
                            ps[:, :],
                            w8_sb[:, 1, :, dt_ * P:(dt_ + 1) * P],
                            s8_sb[:, :, nchk * 512:(nchk + 1) * 512],
                            start=True, stop=True, perf_mode=DR)
                        nc.vector.tensor_scalar_add(
                            k_sb[:, dt_, nchk * 512:(nchk + 1) * 512], ps[:, :],
                            bias[:, 2 + dt_:3 + dt_])
                # v pre-transposed: stationary src-tile, moving WvT
                for mt in range(NMT):
                    ps = pp.tile([P, D], F32, tag="ps")
                    nc.tensor.matmul(
                        ps[:, :],
                        s8_sb[:, :, mt * P:(mt + 1) * P],
                        w8_sb[:, 2, :, :],
                        start=True, stop=True, perf_mode=DR)
                    nc.vector.tensor_add(
                        vt_sb[:, mt, :, :].rearrange("p h d -> p (h d)"),
                        ps[:, :], bvb[:, :])

            # ---- phase 2: attention + per-chunk MLP ----
            with (
                tc.tile_pool(name="pscore", bufs=2, space="PSUM") as pscore,
                tc.tile_pool(name="pmsg", bufs=1, space="PSUM") as pmsg,
                tc.tile_pool(name="pden", bufs=1, space="PSUM") as pden,
                tc.tile_pool(name="edgep", bufs=1) as edgep,
                tc.tile_pool(name="up", bufs=2) as up,
                tc.tile_pool(name="uep", bufs=2) as uep,
                tc.tile_pool(name="accp", bufs=2) as accp,
                tc.tile_pool(name="rdp", bufs=2) as rdp,
                tc.tile_pool(name="outp", bufs=2) as outp,
            ):
                edge_tiles = []
                for c in range(2):
                    edge_t = edgep.tile([P, NMT, 512], F16, tag=f"edge{c}")
                    for g in range(4):
                        nc.sync.dma_start(
                            out=edge_t[:, 4 * g:4 * g + 4, :],
                            in_=edgeT[4 * g * P:4 * (g + 1) * P,
                                      c * 512:(c + 1) * 512].rearrange(
                                          "(t p) n -> p t n", p=P))
                    edge_tiles.append(edge_t)
                nc.sync.dma_start(
                    out=wx_sb[:, 0:W16_COLS],
                    in_=wpk[:, :])
                nc.sync.dma_start(
                    out=wx_sb[:, W16_COLS:],
                    in_=xpk[:, :])

                def emit_scores(c, h):
                    """Score burst + exp + den partials + edge muls for (c,h).
                    Returns state for the deferred msg chain."""
                    edge_t = edge_tiles[c]
                    hb, ht = HD * (h % 2), h // 2
                    qh = q_sb[hb:hb + HD, ht, c * 512:(c + 1) * 512]
                    u = up.tile([P, NMT, 512], F16)
                    ue = uep.tile([P, NMT, 512], F16)
                    acc3 = accp.tile([P, 3, 512], F16, tag="acc3")
                    acc = accp.tile([P, 512], F16, tag="acctop")
                    for gi, (g0, gn) in enumerate(GROUPS):
                        ps = pscore.tile([P, 3, 512], F32, tag="ps2")
                        for j in range(gn):
                            mt = g0 + j
                            nc.tensor.matmul(
                                ps[:, j, :],
                                k_sb[hb:hb + HD, ht, mt * P:(mt + 1) * P],
                                qh, start=True, stop=True)
                        nc.scalar.activation(
                            u[:, g0:g0 + gn, :], ps[:, 0:gn, :],
                            AF.Exp, scale=0.125)
                        if ablate == 'scores':
                            continue
                        # group-wise denominator accumulation (serial chain)
                        if gi == 1:
                            nc.vector.tensor_add(acc3[:, :, :], u[:, 0:3, :],
                                                 u[:, 3:6, :])
                        elif gi in (2, 3, 4):
                            nc.vector.tensor_add(acc3[:, :, :], acc3[:, :, :],
                                                 u[:, g0:g0 + 3, :])
                        elif gi == 5:
                            nc.vector.tensor_add(acc[:, :], acc3[:, 0, :],
                                                 acc3[:, 1, :])
                            nc.vector.tensor_add(acc[:, :], acc[:, :],
                                                 acc3[:, 2, :])
                            nc.vector.tensor_add(acc[:, :], acc[:, :],
                                                 u[:, 15, :])
                        # group-wise edge multiply (one op per group)
                        eng = nc.gpsimd if gi == POOL_GROUP else nc.vector
                        eng.tensor_mul(ue[:, g0:g0 + gn, :],
                                       u[:, g0:g0 + gn, :],
                                       edge_t[:, g0:g0 + gn, :])
                    return (c, h, ue, acc)

                def emit_msg(state):
                    """Uninterrupted msg chain + normalize for a head-chunk."""
                    c, h, ue, acc = state
                    hb, ht = HD * (h % 2), h // 2
                    mps = pmsg.tile([HD, 512], F32, tag="msg")
                    dnb = pden.tile([P, 512], F32, tag="dnb")
                    for mt in range(NMT):
                        nc.tensor.matmul(
                            mps[:, :], vt_sb[:, mt, h, :], ue[:, mt, :],
                            start=(mt == 0), stop=(mt == NMT - 1))
                    nc.tensor.matmul(dnb[0:1, :], ones16[:, :], acc[:, :],
                                     start=True, stop=True)
                    rden = rdp.tile([1, 512], F16, tag="rden")
                    with nc.allow_low_precision("fp16 reciprocal of den"):
                        nc.vector.reciprocal(rden[:, :], dnb[0:1, :])
                    nc.tensor.matmul(dnb[HD:2 * HD, :], ones_row[:, :],
                                     rden[:, :], start=True, stop=True,
                                     skip_group_check=True)
                    rdbc = rdp.tile([HD, 512], F16, tag="rdbc")
                    nc.vector.tensor_copy(rdbc[:, :], dnb[HD:2 * HD, :])
                    nc.vector.tensor_mul(
                        msg_sb[hb:hb + HD, ht, c * 512:(c + 1) * 512],
                        mps[:, :], rdbc[:, :])

                def emit_mlp(c):
                    r = slice(c * 512, (c + 1) * 512)
                    for dt_ in range(2):
                        ps = pscore.tile([P, 512], F32, tag="ps2")
                        for kk in range(2):
                            nc.tensor.matmul(
                                ps[:, :],
                                wm_sb[:, kk, dt_ * P:(dt_ + 1) * P],
                                msg_sb[:, kk, r],
                                start=(kk == 0), stop=(kk == 1))
                        nc.vector.tensor_scalar_add(
                            msg2_sb[:, dt_, r], ps[:, :],
                            bias[:, 6 + dt_:7 + dt_])
                    for dt_ in range(4):
                        ps = pscore.tile([P, 512], F32, tag="ps2")
                        for kk in range(2):
                            nc.tensor.matmul(
                                ps[:, :],
                                w1a_sb[:, kk, dt_ * P:(dt_ + 1) * P],
                                x_sb[:, kk, r],
                                start=(kk == 0), stop=False)
                        for kk in range(2):
                            nc.tensor.matmul(
                                ps[:, :],
                                w1b_sb[:, kk, dt_ * P:(dt_ + 1) * P],
                                msg2_sb[:, kk, r],
                                start=False, stop=(kk == 1))
                        nc.vector.tensor_scalar(
                            h1_sb[:, dt_, r], ps[:, :],
                            bias[:, 8 + dt_:9 + dt_], 0.0,
                            op0=mybir.AluOpType.add,
                            op1=mybir.AluOpType.max)
                    for dt_ in range(2):
                        ps = pscore.tile([P, 512], F32, tag="ps2")
                        for kk in range(4):
                            nc.tensor.matmul(
                                ps[:, :],
                                w2_sb[:, kk, dt_ * P:(dt_ + 1) * P],
                                h1_sb[:, kk, r],
                                start=(kk == 0), stop=(kk == 3))
                        oc = outp.tile([P, 512], F32)
                        nc.vector.tensor_scalar_add(
                            oc[:, :], ps[:, :], bias[:, 12 + dt_:13 + dt_])
                        nc.sync.dma_start(
                            out=out[dt_ * P:(dt_ + 1) * P, r], in_=oc[:, :])

                # software-pipelined: score burst for (c,h), then the
                # UNINTERRUPTED msg chain of the previous head-chunk
                do_msg = ablate in ('full', 'nomlp')
                do_mlp = ablate == 'full'
                # msg/normalize of hc i-1 is emitted BEFORE the score burst
                # of hc i so its DVE tail (recip/final) is queued ahead of
                # S(i)'s DVE work -- the PE bcast never waits behind it
                pending = None
                for c in range(2):
                    for h in range(H):
                        if pending is not None:
                            if do_msg:
                                emit_msg(pending)
                            if pending[1] == H - 1 and do_mlp:
                                emit_mlp(pending[0])
                        pending = (c, h)
                        pending = emit_scores(c, h)
                if do_msg:
                    emit_msg(pending)
                if do_mlp:
                    emit_mlp(1)
                if not do_mlp:
                    # token output so the program still writes `out`
                    oc = outp.tile([P, 512], F32)
                    nc.vector.memset(oc, 1.0)
                    nc.sync.dma_start(out=out[0:P, 0:512], in_=oc[:, :])
        if _loop is not None:
            _loop.__exit__(None, None, None)
    nc.finalize()
    return nc


def _pack_rows(a, nk):
    """[nk*128, C] -> [128, nk*C], k-tile-major per partition."""
    c = a.shape[1]
    return np.ascontiguousarray(
        a.reshape(nk, P, c).transpose(1, 0, 2).reshape(P, nk * c))


def prepare_in_maps(inputs):
    f16 = np.float16
    f8 = ml_dtypes.float8_e4m3
    x = np.asarray(inputs["x"], np.float32)
    source = np.asarray(inputs["source"], np.float32)
    edge = np.asarray(inputs["edge"], np.float32)
    Wq, bq = np.asarray(inputs["Wq"], np.float32), np.asarray(inputs["bq"], np.float32)
    Wk, bk = np.asarray(inputs["Wk"], np.float32), np.asarray(inputs["bk"], np.float32)
    Wv, bv = np.asarray(inputs["Wv"], np.float32), np.asarray(inputs["bv"], np.float32)
    Wm, bm = np.asarray(inputs["Wm"], np.float32), np.asarray(inputs["bm"], np.float32)
    W1, b1 = np.asarray(inputs["W1"], np.float32), np.asarray(inputs["b1"], np.float32)
    W2, b2 = np.asarray(inputs["W2"], np.float32), np.asarray(inputs["b2"], np.float32)

    # head-major channel permutation: j = h*64+i  <->  torch c = i*4+h
    perm = np.array([(j % HD) * H + j // HD for j in range(D)])

    wpk8 = np.concatenate([
        _pack_rows(Wq[perm].T.astype(f8), 2),
        _pack_rows(Wk[perm].T.astype(f8), 2),
        _pack_rows(Wv[perm].T.astype(f8), 2),
    ], axis=1)
    wpk = np.concatenate([
        _pack_rows(W1[:, :D].T.astype(f16), 2),
        _pack_rows(Wm[:, perm].T.astype(f16), 2),
        _pack_rows(W1[:, D:].T.astype(f16), 2),
        _pack_rows(W2.T.astype(f16), 4),
    ], axis=1)
    bpk = np.stack([
        bq[perm][:P], bq[perm][P:], bk[perm][:P], bk[perm][P:],
        bv[perm][:P], bv[perm][P:], bm[:P], bm[P:],
        b1[:P], b1[P:2 * P], b1[2 * P:3 * P], b1[3 * P:],
        b2[:P], b2[P:],
    ], axis=1).astype(np.float32)
    bpk = np.ascontiguousarray(bpk)
    bvbc_arr = np.ascontiguousarray(
        np.broadcast_to(bv[perm][None, :], (P, D)).astype(f16))

    shared = {"wpk8": wpk8, "wpk": wpk, "bpk": bpk, "bvbc": bvbc_arr}
    in_maps = []
    for c in range(NCORES):
        b, half = c // 2, c % 2
        sl = slice(half * NQ, (half + 1) * NQ)
        in_maps.append({
            "xpk8": _pack_rows(x[b, :, sl].astype(f8), 2),
            "xpk": _pack_rows(x[b, :, sl].astype(f16), 2),
            "spk8": _pack_rows(source[b].astype(f8), 2),
            "edgeT": np.ascontiguousarray(edge[b, sl, :].T.astype(f16)),
            **shared,
        })
    return in_maps


def kernel(**inputs) -> np.ndarray:
    global LAST_RESULTS
    in_maps = prepare_in_maps(inputs)
    nc = build_program()
    LAST_RESULTS = bass_utils.run_bass_kernel_spmd(
        nc, in_maps, core_ids=list(range(NCORES)),
        trace=os.environ.get("BASS_KERNEL_TRACE", "0") == "1",
    )

    y = np.empty((B, D, N), np.float32)
    for c in range(NCORES):
        b, half = c // 2, c % 2
        y[b, :, half * NQ:(half + 1) * NQ] = LAST_RESULTS.results[c]["out"]
    return y
